# revision 1
# baseline (speedup 1.0000x reference)
"""GCN layer (gnn_message_passing) on 8 Trainium2 NeuronCores.

Reference computation:
    deg = segment_sum(ones, hs)              # in-degree of each node (rows hs)
    s   = deg ** -0.5
    agg[h] = sum over edges (h, t) of s[t] * feats[t]
    out = relu((s[:, None] * agg) @ W.T)

Distribution strategy (per the sharding hint):
  * Nodes are sharded across the 8 cores (12500 each); edges are partitioned
    by destination (hs) so the segment-sum is core-local.
  * feats is replicated to every core's HBM; each core gathers the source rows
    it needs with indirect DMA, 128 rows (one per SBUF partition) per call.
  * The per-edge normalization s[t]*s[h] is folded into a per-edge weight, so
    the whole SpMM becomes, per 128-node output group:
        agg_T[f, s] = sum_k  G_k[e, f]^T @ S_k[e, s]
    where G_k is a 128-edge block of gathered rows and S_k is a one-hot
    selection matrix scaled by the edge weight, built on the fly by one DVE
    tensor_scalar op:  S_k[e, s] = (iota[s] == hs_off[e]) * w[e].
  * Host-side preprocessing is integer-only bookkeeping: bucket edges by
    destination group (counting sort), pad each group to B blocks of 128 edge
    slots, and look up integer degree products. All floating-point math
    (rsqrt, scaling, SpMM, linear, relu) runs on device.

Every core runs the identical program (SPMD); only the per-core index/meta
data differs.
"""

import numpy as np

import concourse.bacc as bacc
import concourse.bass as bass
import concourse.mybir as mybir
import concourse.tile as tile
from concourse import bass_utils

N_N = 100000
N_E = 1600000
D = 128
N_CORES = 8
NPC = N_N // N_CORES  # nodes per core
P = 128
GPC = -(-NPC // P)  # 128-node groups per core

F32 = mybir.dt.float32
I32 = mybir.dt.int32


def prep(edges, n_nodes=N_N, n_cores=N_CORES, npc=NPC, gpc=GPC):
    """Integer-only host preprocessing: bucket edges by destination group.

    Returns (B, metas) where B is blocks-per-group and metas[c] is the int32
    [P, gpc*3B] per-core meta array; per-group columns are
    [ts_idx (B) | hs_off as f32 bits (B) | deg_t*deg_h as f32 bits (B)].
    """
    hs = np.asarray(edges[0], dtype=np.int64)
    ts = np.asarray(edges[1], dtype=np.int64)
    n_e = hs.shape[0]
    deg = np.bincount(hs, minlength=n_nodes)

    core = hs // npc
    local = hs - core * npc
    g_local = local // P
    off = local - g_local * P
    gg = core * gpc + g_local

    # Sort by destination group, then by source within each group: the
    # secondary source order gives the row gathers DRAM locality (~2x).
    order = np.lexsort((ts, gg))
    gg_s = gg[order]
    ts_s = ts[order]
    off_s = off[order]
    degprod_s = (deg[ts_s] * deg[hs[order]]).astype(np.float32)

    tg = n_cores * gpc
    counts = np.bincount(gg, minlength=tg)
    B = max(1, int(-(-counts.max() // P)))
    S = B * P

    starts = np.zeros(tg + 1, np.int64)
    np.cumsum(counts, out=starts[1:])
    pos = np.arange(n_e, dtype=np.int64) - starts[gg_s]
    flat = gg_s * S + pos

    idx_pad = np.zeros(tg * S, np.int32)
    off_pad = np.full(tg * S, 255.0, np.float32)
    dp_pad = np.ones(tg * S, np.float32)
    idx_pad[flat] = ts_s
    off_pad[flat] = off_s
    dp_pad[flat] = degprod_s

    idx_pbm = idx_pad.reshape(tg, B, P).transpose(0, 2, 1)
    off_pbm = off_pad.reshape(tg, B, P).transpose(0, 2, 1)
    dp_pbm = dp_pad.reshape(tg, B, P).transpose(0, 2, 1)

    meta = np.empty((tg, P, 3 * B), np.int32)
    meta[:, :, :B] = idx_pbm
    meta[:, :, B : 2 * B] = off_pbm.view(np.int32)
    meta[:, :, 2 * B :] = dp_pbm.view(np.int32)
    metas = (
        meta.reshape(n_cores, gpc, P, 3 * B)
        .transpose(0, 2, 1, 3)
        .reshape(n_cores, P, gpc * 3 * B)
        .copy()
    )
    return B, metas


def build_gcn(B, n_nodes=N_N, gpc=GPC, g_bufs=12, s_bufs=8):
    """Build the SPMD Bass program for one core (all cores identical)."""
    nc = bacc.Bacc(
        "TRN2",
        target_bir_lowering=False,
        debug=False,
        enable_asserts=False,
        num_devices=N_CORES,
    )
    feats_d = nc.dram_tensor("feats", [n_nodes, D], F32, kind="ExternalInput")
    meta_d = nc.dram_tensor("meta", [P, gpc * 3 * B], I32, kind="ExternalInput")
    wt_d = nc.dram_tensor("wt", [D, D], F32, kind="ExternalInput")
    iota_d = nc.dram_tensor("iota", [P, P], F32, kind="ExternalInput")
    out_d = nc.dram_tensor("out", [gpc * P, D], F32, kind="ExternalOutput")

    with tile.TileContext(nc) as tc:
        with (
            tc.tile_pool(name="const", bufs=1) as cpool,
            tc.tile_pool(name="gpool", bufs=g_bufs) as gpool,
            tc.tile_pool(name="spool", bufs=s_bufs) as spool,
            tc.tile_pool(name="wpool", bufs=3) as wpool,
            tc.tile_pool(name="mpool", bufs=3) as mpool,
            tc.tile_pool(name="opool", bufs=3) as opool,
            tc.tile_pool(name="psA", bufs=2, space="PSUM") as psA,
            tc.tile_pool(name="psB", bufs=2, space="PSUM") as psB,
        ):
            meta_sb = cpool.tile([P, gpc * 3 * B], I32)
            nc.sync.dma_start(meta_sb[:], meta_d[:])
            wt_sb = cpool.tile([P, P], F32)
            nc.sync.dma_start(wt_sb[:], wt_d[:])
            iota_sb = cpool.tile([P, P], F32)
            nc.sync.dma_start(iota_sb[:], iota_d[:])

            for g in range(gpc):
                base = g * 3 * B
                # Per-edge weight w = (deg_t * deg_h) ** -0.5.
                rec = wpool.tile([P, B], F32, tag="rec")
                nc.vector.reciprocal(
                    rec[:], meta_sb[:, base + 2 * B : base + 3 * B].bitcast(F32)
                )
                wsb = wpool.tile([P, B], F32, tag="w")
                nc.scalar.sqrt(wsb[:], rec[:])

                agg = psA.tile([P, P], F32, tag="agg")
                for k in range(B):
                    # Gather 128 source rows (one per partition).
                    Gk = gpool.tile([P, P], F32, tag="G")
                    nc.gpsimd.indirect_dma_start(
                        out=Gk[:],
                        out_offset=None,
                        in_=feats_d[:],
                        in_offset=bass.IndirectOffsetOnAxis(
                            ap=meta_sb[:, base + k : base + k + 1], axis=0
                        ),
                    )
                    St = spool.tile([P, P], F32, tag="S")
                    nc.vector.tensor_scalar(
                        out=St[:],
                        in0=iota_sb[:],
                        scalar1=meta_sb[:, base + B + k : base + B + k + 1].bitcast(F32),
                        scalar2=wsb[:, k : k + 1],
                        op0=mybir.AluOpType.is_equal,
                        op1=mybir.AluOpType.mult,
                    )
                    nc.tensor.matmul(
                        agg[:],
                        lhsT=Gk[:],
                        rhs=St[:],
                        start=(k == 0),
                        stop=(k == B - 1),
                    )
                # agg is [feat, seg]; linear layer contracts over feat.
                msgt = mpool.tile([P, P], F32, tag="msgt")
                nc.vector.tensor_copy(msgt[:], agg[:])
                out2 = psB.tile([P, P], F32, tag="out2")
                nc.tensor.matmul(
                    out2[:], lhsT=msgt[:], rhs=wt_sb[:], start=True, stop=True
                )
                osb = opool.tile([P, P], F32, tag="osb")
                nc.scalar.activation(
                    osb[:], out2[:], mybir.ActivationFunctionType.Relu
                )
                nc.sync.dma_start(out_d[g * P : (g + 1) * P, :], osb[:])

    nc.compile()
    return nc


_CACHE = {}


def _run(feats_n, edges, weight, trace=False):
    feats = np.ascontiguousarray(np.asarray(feats_n, dtype=np.float32))
    weight = np.asarray(weight, dtype=np.float32)
    B, metas = prep(edges)

    if B not in _CACHE:
        _CACHE[B] = build_gcn(B)
    nc = _CACHE[B]

    wt = np.ascontiguousarray(weight.T)
    iota = np.ascontiguousarray(
        np.broadcast_to(np.arange(P, dtype=np.float32), (P, P))
    )
    in_maps = [
        {"feats": feats, "meta": metas[c], "wt": wt, "iota": iota}
        for c in range(N_CORES)
    ]
    res = bass_utils.run_bass_kernel_spmd(
        nc, in_maps, core_ids=list(range(N_CORES)), trace=trace
    )
    out = np.concatenate(
        [res.results[c]["out"][:NPC] for c in range(N_CORES)], axis=0
    )
    return np.ascontiguousarray(out, dtype=np.float32), res


def kernel(feats_n, edges, weight):
    out, _ = _run(feats_n, edges, weight)
    return out



# revision 8
# speedup vs baseline: 1.3102x; 1.3102x over previous
"""GCN layer (gnn_message_passing) on 8 Trainium2 NeuronCores.

Reference computation:
    deg = segment_sum(ones, hs)              # in-degree of each node (rows hs)
    s   = deg ** -0.5
    agg[h] = sum over edges (h, t) of s[t] * feats[t]
    out = relu((s[:, None] * agg) @ W.T)

Distribution strategy (per the sharding hint):
  * Nodes are sharded across the 8 cores (12500 each); edges are partitioned
    by destination (hs) so the segment-sum is core-local. feats is replicated
    to every core's HBM.
  * Destinations are processed in groups of 256; the per-edge normalization
    s[t]*s[h] is folded into a per-edge weight so the SpMM becomes, per
    128-edge block:  agg[f, d] += G_k[e, f]^T @ S_k[e, d], where G_k holds the
    gathered source rows and S_k is a one-hot selection matrix scaled by the
    edge weight, built by one DVE tensor_scalar op.
  * Source rows are fetched with big batched `dma_gather` instructions (a few
    thousand rows each) instead of per-block indirect DMAs - the SWDGE ~1us
    fixed cost per DMA instruction dominated the previous version. dma_gather
    indices are int16, so sources are bucketed into 4 chunks of 25000 rows.
  * Gathered rows are converted fp32->fp16 on the Scalar/Vector engines so the
    PE runs at 16-bit rate; accumulation stays fp32 in PSUM.
  * The program is SPMD (one BIR for all cores), so per-(group, chunk) block
    counts are baked as the max over cores (~10% padding).
  * Host-side preprocessing is integer-only bookkeeping (bucketing, padding,
    int16 index packing, integer degree products). All floating-point math
    (rsqrt, scaling, SpMM, linear, relu) runs on device.
"""

import numpy as np

import concourse.bacc as bacc
import concourse.bass as bass
import concourse.mybir as mybir
import concourse.tile as tile
from concourse import bass_utils

N_N = 100000
N_E = 1600000
D = 128
N_CORES = 8
NPC = N_N // N_CORES  # nodes per core
P = 128
DG = 256  # destination-group width
GPC = -(-NPC // DG)  # dest groups per core (49)
NCH = 4  # source chunks (int16 gather indices)
CH = N_N // NCH  # chunk size (25000)
SBW = 4  # dest groups per gather superblock
NSB = -(-GPC // SBW)  # superblocks per core (13)
MAXBLK = 64  # max 128-row blocks (8192 rows) per dma_gather instruction

F32 = mybir.dt.float32
F16 = mybir.dt.float16
I16 = mybir.dt.int16

OFF_PAD = 300.0  # is_equal(iota 0..255, 300) is always false


def prep(edges):
    """Integer-only host preprocessing.

    Buckets edges by (dest core, dest group of 256, source chunk of 25000),
    pads each bucket to a shared (max-over-cores) number of 128-edge blocks,
    and packs per-slot metadata:
      - idx16: int16 gather indices, dma_gather layout ([128, slots/16],
        idx i at [i%16, i//16], replicated over the 8 16-partition groups)
      - off32: fp32 destination offset within the group (pad slots: 300)
      - dp32:  fp32 integer-valued degree product deg[t]*deg[h] (pad: 1)
    Returns (nblk, idx16, off32, dp32) where nblk[g, c] is the shared
    per-bucket block count.
    """
    hs = np.asarray(edges[0], dtype=np.int64)
    ts = np.asarray(edges[1], dtype=np.int64)
    n_e = hs.shape[0]
    deg = np.bincount(hs, minlength=N_N)

    core = hs // NPC
    local = hs - core * NPC
    g = local // DG
    off = local - g * DG
    ch = ts // CH
    tloc = ts - ch * CH

    # bucket order: core-major, then superblock, then chunk, then group
    # within superblock - matching the slot order the device consumes.
    sb = g // SBW
    gw = g - sb * SBW
    bucket = ((core * NSB + sb) * NCH + ch) * SBW + gw
    nbkt = N_CORES * NSB * NCH * SBW

    order = np.lexsort((ts, bucket))
    bkt_s = bucket[order]
    tloc_s = tloc[order]
    off_s = off[order]
    dp_s = (deg[ts[order]] * deg[hs[order]]).astype(np.float32)

    counts = np.bincount(bkt_s, minlength=nbkt).reshape(N_CORES, NSB, NCH, SBW)
    # shared (SPMD) block counts: max over cores per (group, chunk)
    nblk_sc = -(-counts.max(axis=0) // P)  # [NSB, NCH, SBW]
    nblk = np.zeros((GPC, NCH), np.int64)
    for s in range(NSB):
        for c in range(NCH):
            for w in range(SBW):
                gidx = s * SBW + w
                if gidx < GPC:
                    nblk[gidx, c] = nblk_sc[s, c, w]
    slots_per_bucket = (nblk_sc * P).astype(np.int64)  # [NSB, NCH, SBW]
    tot_slots = int(slots_per_bucket.sum())
    tot_blk = tot_slots // P

    # slot start of each bucket (identical across cores)
    flat_slots = slots_per_bucket.reshape(-1)
    starts1 = np.zeros(NSB * NCH * SBW + 1, np.int64)
    np.cumsum(flat_slots, out=starts1[1:])
    # per-core slot position of each edge
    within = bkt_s % (NSB * NCH * SBW)
    counts_flat = np.bincount(bkt_s, minlength=nbkt)
    bstarts = np.zeros(nbkt + 1, np.int64)
    np.cumsum(counts_flat, out=bstarts[1:])
    pos_in_bucket = np.arange(n_e, dtype=np.int64) - bstarts[bkt_s]
    slot = starts1[within] + pos_in_bucket

    core_s = bkt_s // (NSB * NCH * SBW)

    idx_pad = np.zeros((N_CORES, tot_slots), np.int16)
    off_pad = np.full((N_CORES, tot_slots), OFF_PAD, np.float32)
    dp_pad = np.ones((N_CORES, tot_slots), np.float32)
    idx_pad[core_s, slot] = tloc_s.astype(np.int16)
    off_pad[core_s, slot] = off_s.astype(np.float16)
    dp_pad[core_s, slot] = dp_s

    # dma_gather index layout: [128, tot_slots/16], idx i at [i%16, i//16],
    # the 16-row pattern replicated 8x across partitions.
    idx16 = np.ascontiguousarray(
        np.tile(
            idx_pad.reshape(N_CORES, tot_slots // 16, 16).transpose(0, 2, 1),
            (1, 8, 1),
        )
    )
    # per-block scalar layout: [128, tot_blk], slot b*128+p at [p, b]
    off32 = np.ascontiguousarray(
        off_pad.reshape(N_CORES, tot_blk, P).transpose(0, 2, 1)
    )
    dp32 = np.ascontiguousarray(
        dp_pad.reshape(N_CORES, tot_blk, P).transpose(0, 2, 1)
    )
    return nblk, idx16, off32, dp32


def build_gcn(nblk, g_bufs=3, s_bufs=8):
    """Build the SPMD Bass program (identical for all cores).

    nblk[g, c] = number of 128-edge blocks for dest group g, source chunk c.
    """
    nblk = np.asarray(nblk)
    tot_blk = int(nblk.sum())
    tot_slots = tot_blk * P

    nc = bacc.Bacc(
        "TRN2",
        target_bir_lowering=False,
        debug=False,
        enable_asserts=False,
        num_devices=N_CORES,
    )
    feats_cd = [
        nc.dram_tensor(f"feats{c}", [CH, D], F32, kind="ExternalInput")
        for c in range(NCH)
    ]
    idx_d = nc.dram_tensor("idx16", [P, tot_slots // 16], I16, kind="ExternalInput")
    off_d = nc.dram_tensor("off32", [P, tot_blk], F32, kind="ExternalInput")
    dp_d = nc.dram_tensor("dp32", [P, tot_blk], F32, kind="ExternalInput")
    wt_d = nc.dram_tensor("wt", [D, D], F32, kind="ExternalInput")
    iota_d = nc.dram_tensor("iota", [P, DG], F16, kind="ExternalInput")
    out_d = nc.dram_tensor("out", [GPC * DG, D], F32, kind="ExternalOutput")

    # block column start per (g, c), in slot order (sb, c, g-within-sb)
    blkcol = np.zeros((GPC, NCH), np.int64)
    col = 0
    for s in range(NSB):
        for c in range(NCH):
            for w in range(SBW):
                gidx = s * SBW + w
                if gidx < GPC:
                    blkcol[gidx, c] = col
                    col += nblk[gidx, c]
    assert col == tot_blk

    with tile.TileContext(nc) as tc:
        with (
            tc.tile_pool(name="const", bufs=1) as cpool,
            tc.tile_pool(name="prep", bufs=1) as ppool,
            tc.tile_pool(name="gpool", bufs=g_bufs) as gpool,
            tc.tile_pool(name="gbf", bufs=8) as gbfpool,
            tc.tile_pool(name="spool", bufs=s_bufs) as spool,
            tc.tile_pool(name="mpool", bufs=3) as mpool,
            tc.tile_pool(name="opool", bufs=4) as opool,
            tc.tile_pool(name="psA", bufs=6, space="PSUM") as psA,
            tc.tile_pool(name="psB", bufs=2, space="PSUM") as psB,
        ):
            idx_sb = cpool.tile([P, tot_slots // 16], I16, tag="idx")
            nc.sync.dma_start(idx_sb[:], idx_d[:])
            off_sb = cpool.tile([P, tot_blk], F32, tag="off")
            nc.sync.dma_start(off_sb[:], off_d[:])
            dp_sb = ppool.tile([P, tot_blk], F32, tag="dp")
            nc.sync.dma_start(dp_sb[:], dp_d[:])
            iota_sb = cpool.tile([P, DG], F16, tag="iota")
            nc.sync.dma_start(iota_sb[:], iota_d[:])
            wt_sb = ppool.tile([P, P], F32, tag="wt")
            nc.sync.dma_start(wt_sb[:], wt_d[:])

            # per-edge weight w = (deg_t * deg_h) ** -0.5, in fp16
            rec_sb = ppool.tile([P, tot_blk], F32, tag="rec")
            nc.vector.reciprocal(rec_sb[:], dp_sb[:])
            w_sb = cpool.tile([P, tot_blk], F32, tag="w")
            nc.scalar.sqrt(w_sb[:], rec_sb[:])
            wt16_sb = cpool.tile([P, P], F16, tag="wt16")
            nc.scalar.activation(
                wt16_sb[:], wt_sb[:], mybir.ActivationFunctionType.Copy
            )

            cvt = 0  # alternate fp32->fp16 converts between scalar/vector
            for s in range(NSB):
                glist = [s * SBW + w for w in range(SBW) if s * SBW + w < GPC]
                aggs = {}
                done = {}
                for g in glist:
                    if int(nblk[g].sum()) > 0:
                        aggs[g] = psA.tile([P, DG], F32, tag="agg", name="agg")
                        done[g] = 0
                for c in range(NCH):
                    nb = int(sum(nblk[g, c] for g in glist))
                    if nb == 0:
                        continue
                    col0 = int(blkcol[glist[0], c])
                    gtile = gpool.tile([P, nb, P], F32, tag="G")
                    # split into <=MAXBLK-block pieces (descriptor-ring cap)
                    b0 = 0
                    while b0 < nb:
                        b1 = min(b0 + MAXBLK, nb)
                        slot0 = (col0 + b0) * P
                        slot1 = (col0 + b1) * P
                        nc.gpsimd.dma_gather(
                            gtile[:, b0:b1, :],
                            feats_cd[c][:],
                            idx_sb[:, slot0 // 16 : slot1 // 16],
                            (b1 - b0) * P,
                            (b1 - b0) * P,
                            D,
                            single_packet=False,
                        )
                        b0 = b1
                    for g in glist:
                        tot_g = int(nblk[g].sum())
                        for k in range(int(nblk[g, c])):
                            col = int(blkcol[g, c]) + k
                            gk16 = gbfpool.tile([P, P], F16, tag="gk")
                            src = gtile[:, col - col0, :]
                            if cvt % 2 == 0:
                                nc.scalar.activation(
                                    gk16[:], src, mybir.ActivationFunctionType.Copy
                                )
                            else:
                                nc.vector.tensor_copy(gk16[:], src)
                            cvt += 1
                            st = spool.tile([P, DG], F16, tag="S")
                            nc.vector.tensor_scalar(
                                out=st[:],
                                in0=iota_sb[:],
                                scalar1=off_sb[:, col : col + 1],
                                scalar2=w_sb[:, col : col + 1],
                                op0=mybir.AluOpType.is_equal,
                                op1=mybir.AluOpType.mult,
                            )
                            nc.tensor.matmul(
                                aggs[g][:],
                                lhsT=gk16[:],
                                rhs=st[:],
                                start=(done[g] == 0),
                                stop=(done[g] == tot_g - 1),
                            )
                            done[g] += 1
                # linear + relu on the two 128-dest halves of each group
                for g in glist:
                    if g not in aggs:
                        continue
                    msgt = mpool.tile([P, DG], F16, tag="msgt")
                    nc.vector.tensor_copy(msgt[:], aggs[g][:])
                    for j in range(DG // P):
                        out2 = psB.tile([P, P], F32, tag="out2")
                        nc.tensor.matmul(
                            out2[:],
                            lhsT=msgt[:, j * P : (j + 1) * P],
                            rhs=wt16_sb[:],
                            start=True,
                            stop=True,
                        )
                        osb = opool.tile([P, P], F32, tag="osb")
                        nc.scalar.activation(
                            osb[:], out2[:], mybir.ActivationFunctionType.Relu
                        )
                        nc.sync.dma_start(
                            out_d[g * DG + j * P : g * DG + (j + 1) * P, :], osb[:]
                        )

    nc.compile()
    return nc


_CACHE = {}


def _run(feats_n, edges, weight, trace=False):
    feats = np.ascontiguousarray(np.asarray(feats_n, dtype=np.float32))
    weight = np.asarray(weight, dtype=np.float32)
    nblk, idx16, off32, dp32 = prep(edges)

    key = nblk.tobytes()
    if key not in _CACHE:
        _CACHE[key] = build_gcn(nblk)
    nc = _CACHE[key]

    wt = np.ascontiguousarray(weight.T)
    iota = np.ascontiguousarray(
        np.broadcast_to(np.arange(DG, dtype=np.float16), (P, DG))
    )
    in_maps = [
        {
            **{
                f"feats{j}": np.ascontiguousarray(feats[j * CH : (j + 1) * CH])
                for j in range(NCH)
            },
            "idx16": idx16[c],
            "off32": off32[c],
            "dp32": dp32[c],
            "wt": wt,
            "iota": iota,
        }
        for c in range(N_CORES)
    ]
    res = bass_utils.run_bass_kernel_spmd(
        nc, in_maps, core_ids=list(range(N_CORES)), trace=trace
    )
    out = np.concatenate(
        [res.results[c]["out"][:NPC] for c in range(N_CORES)], axis=0
    )
    return np.ascontiguousarray(out, dtype=np.float32), res


def kernel(feats_n, edges, weight):
    out, _ = _run(feats_n, edges, weight)
    return out
